# revision 7
# baseline (speedup 1.0000x reference)
"""Bass/Trainium2 kernel for BNBLinear4bit (NF4 dequant + matmul + bias).

Strategy (8 NeuronCores, tensor-parallel on out_features):
  - each core gets a 512-row shard of codes/absmax/bias, x replicated
  - NF4 dequant on-device via an exact 16-point piecewise-linear basis:
    3 scaled-step terms on DVE (tensor_scalar is_ge*coef @4x) and 12
    amplitude-folded relu ramps on ACT, combined with fp16 adds
  - fp16 matmul (PE full rate), fp32 PSUM accumulation
  - x cast f32->fp16 during SWDGE DMA, transposed on-chip via xbar DMA
  - w transposed via xbar DMA after dequant
  - out = psum + bias (fp32), gathered on host by concatenation
"""
import sys

sys.path.insert(0, "/opt/trn_rl_repo")

import numpy as np

import concourse.bass as bass
import concourse.mybir as mybir
from concourse import bacc
from concourse.bass_utils import run_bass_kernel_spmd
from concourse.tile import TileContext

F16 = mybir.dt.float16
F32 = mybir.dt.float32
I32 = mybir.dt.int32
ALU = mybir.AluOpType
ACTF = mybir.ActivationFunctionType

NF4 = np.array([
    -1.0, -0.6961928009986877, -0.5250730514526367, -0.39491748809814453,
    -0.28444138169288635, -0.18477343022823334, -0.09105003625154495, 0.0,
    0.07958029955625534, 0.16093020141124725, 0.24611230194568634,
    0.33791524171829224, 0.44070982933044434, 0.5626170039176941,
    0.6797559261322021, 1.0], dtype=np.float64)

BLOCKSIZE = 64
N_CORES = 8

# k values whose basis term is a scaled step evaluated on DVE; the rest are
# amplitude-folded relu ramps evaluated on ACT.
STEP_KS = (1, 2, 3)


def _solve_basis():
    """T(c) = K0 + sum_{k in D} a_k*[c>=k] + sum_{k in A} g_k*relu(c-(k-1)),
    solved exactly at the 16 integer codes."""
    c = np.arange(16.0)
    D = list(STEP_KS)
    A = [k for k in range(1, 16) if k not in STEP_KS]
    cols = [np.ones(16)]
    for k in D:
        cols.append((c >= k).astype(float))
    for k in A:
        cols.append(np.maximum(c - (k - 1), 0.0))
    coef = np.linalg.solve(np.stack(cols, axis=1), NF4)
    K0 = float(coef[0])
    terms = []  # (kind, k, coef)
    for i, k in enumerate(D):
        terms.append(("step", k, float(coef[1 + i])))
    for i, k in enumerate(A):
        terms.append(("ramp", k, float(coef[1 + len(D) + i])))
    # ascending |coef| limits fp16 accumulation error; initializers (first
    # two consumed) must be steps or positive ramps so the raw pass output
    # equals the signed term
    terms.sort(key=lambda t: abs(t[2]))
    order = []
    inits = 0
    deferred = []
    for t in terms:
        if inits < 2:
            if t[0] == "step" or t[2] >= 0:
                order.append(t)
                inits += 1
            else:
                deferred.append(t)
        else:
            order.append(t)
    order[2:2] = deferred
    return K0, order


def build_bass(BS, IN, OSH, B_BLK=4):
    """Build the per-core Bass program. All 8 cores run this SPMD on their
    own shard."""
    P = 128
    KT = IN // P              # contraction tiles
    OPT = OSH // P            # o partition-tiles (codes row chunks)
    NBS = BS // P             # bs tiles
    OHW = OSH // 2            # psum free width (one o-half)
    IH = IN // 2              # dequant chunk width (i-half)
    NBLK = NBS // B_BLK
    KH = KT // 2              # k tiles per k-half

    K0, order = _solve_basis()

    nc = bacc.Bacc(trn_type="TRN2")
    x_d = nc.dram_tensor("x", [BS, IN], F32, kind="ExternalInput")
    codes_d = nc.dram_tensor("codes", [OSH, IN], I32, kind="ExternalInput")
    absmax_d = nc.dram_tensor("absmax", [OSH, IN // BLOCKSIZE], F32,
                              kind="ExternalInput")
    bias_d = nc.dram_tensor("bias", [OSH], F32, kind="ExternalInput")
    out_d = nc.dram_tensor("out", [BS, OSH], F32, kind="ExternalOutput")

    with TileContext(nc) as tc:
        with (
            tc.tile_pool(name="wt", bufs=1) as wt_pool,
            tc.tile_pool(name="const", bufs=1) as const_pool,
            tc.tile_pool(name="amax", bufs=1) as amax_pool,
            tc.tile_pool(name="c16", bufs=2) as c16_pool,
            tc.tile_pool(name="srep", bufs=2) as srep_pool,
            tc.tile_pool(name="vterm", bufs=3) as v_pool,
            tc.tile_pool(name="acc1", bufs=2) as acc1_pool,
            tc.tile_pool(name="acc2", bufs=2) as acc2_pool,
            tc.tile_pool(name="wn", bufs=2) as wn_pool,
            tc.tile_pool(name="xnat", bufs=2) as xnat_pool,
            tc.tile_pool(name="xt", bufs=B_BLK + 1) as xt_pool,
            tc.tile_pool(name="osb", bufs=B_BLK + 1) as osb_pool,
            tc.tile_pool(name="psum", bufs=6, space="PSUM") as psum_pool,
        ):
            # bias replicated across partitions (fp32)
            brep = const_pool.tile([P, OSH], F32)
            nc.gpsimd.dma_start(brep[:], bias_d[None, :].broadcast_to([P, OSH]))

            # per-ramp activation bias constants [P, 1]
            rbias = {}
            for (kind, k, w) in order:
                if kind == "ramp":
                    val = float(-(k - 1) * abs(w))
                    t = const_pool.tile([P, 1], F32, tag=f"rb{k}")
                    nc.gpsimd.memset(t[:], val)
                    rbias[k] = t

            # absmax shard, one [P, IN//BLOCKSIZE] tile per o-ptile
            amax = []
            for op in range(OPT):
                t = amax_pool.tile([P, IN // BLOCKSIZE], F32, tag=f"amax{op}")
                nc.sync.dma_start(t[:], absmax_d[op * P:(op + 1) * P, :])
                amax.append(t)

            # w^T, fp16, [P, KT*OSH]; element (p, k*OSH + o) = w[o, k*P + p]
            wT = wt_pool.tile([P, KT * OSH], F16)
            wT3 = wT[:].rearrange("p (k o) -> p k o", k=KT)

            # ---- dequant, phase-ordered to match matmul consumption:
            # phase (ihalf, ohalf): chunks (op in ohalf) x (i in ihalf)
            for ih in range(2):
                for oh in range(2):
                    for opl in range(OPT // 2):
                        op = oh * (OPT // 2) + opl
                        c16 = c16_pool.tile([P, IH], F16)
                        nc.gpsimd.dma_start(
                            c16[:], codes_d[op * P:(op + 1) * P,
                                            ih * IH:(ih + 1) * IH])
                        # scale, replicated 64x along i
                        nb = IH // BLOCKSIZE
                        srep = srep_pool.tile([P, IH], F16)
                        nc.vector.tensor_copy(
                            srep[:].rearrange("p (b r) -> p b r", b=nb),
                            amax[op][:, ih * nb:(ih + 1) * nb][:, :, None]
                            .broadcast_to([P, nb, BLOCKSIZE]),
                        )
                        accs = [None, None]

                        def emit_term(kind, k, w, dst):
                            if kind == "step":
                                nc.vector.tensor_scalar(
                                    dst[:], c16[:], float(k), float(w),
                                    ALU.is_ge, ALU.mult)
                            else:
                                g = abs(w)
                                nc.scalar.activation(
                                    dst[:], c16[:], ACTF.Relu,
                                    bias=rbias[k][:], scale=g)

                        ai = 0
                        for (kind, k, w) in order:
                            if accs[ai % 2] is None:
                                dst = (acc1_pool if ai % 2 == 0 else
                                       acc2_pool).tile([P, IH], F16)
                                emit_term(kind, k, w, dst)
                                accs[ai % 2] = dst
                            else:
                                v = v_pool.tile([P, IH], F16)
                                emit_term(kind, k, w, v)
                                a = accs[ai % 2]
                                if kind == "ramp" and w < 0:
                                    nc.any.tensor_sub(a[:], a[:], v[:])
                                else:
                                    nc.any.tensor_add(a[:], a[:], v[:])
                            ai += 1
                        a1, a2 = accs
                        nc.any.tensor_add(a1[:], a1[:], a2[:])
                        # w = (acc + K0) * scale  -> fp16
                        wn = wn_pool.tile([P, IH], F16)
                        nc.vector.scalar_tensor_tensor(
                            wn[:], a1[:], K0, srep[:], ALU.add, ALU.mult)
                        # transpose into wT[:, ih*KH + kk, op*P + o]
                        nc.sync.dma_start_transpose(
                            wT3[:, ih * KH:(ih + 1) * KH, op * P:(op + 1) * P],
                            wn[:],
                        )

            # ---- matmul: blocks of B_BLK bs-tiles; per block sweep
            # (kh, oh) in dequant phase order, accumulate in out_sb
            for blk in range(NBLK):
                xts = []
                for b in range(B_BLK):
                    bs = blk * B_BLK + b
                    xn = xnat_pool.tile([P, IN], F16)
                    nc.gpsimd.dma_start(xn[:], x_d[bs * P:(bs + 1) * P, :])
                    xt = xt_pool.tile([P, KT * P], F16)
                    nc.sync.dma_start_transpose(
                        xt[:].rearrange("p (k b) -> p k b", k=KT), xn[:])
                    xts.append(xt[:].rearrange("p (k b) -> p k b", k=KT))
                osbs = [osb_pool.tile([P, OSH], F32, tag="osb", name="osb") for _ in range(B_BLK)]
                for ih in range(2):
                    for oh in range(2):
                        for b in range(B_BLK):
                            ps = psum_pool.tile([P, OHW], F32)
                            for kk in range(KH):
                                k = ih * KH + kk
                                nc.tensor.matmul(
                                    ps[:],
                                    xts[b][:, k, :],
                                    wT3[:, k, oh * OHW:(oh + 1) * OHW],
                                    start=(kk == 0), stop=(kk == KH - 1))
                            dst = osbs[b][:, oh * OHW:(oh + 1) * OHW]
                            if ih == 0:
                                nc.any.tensor_add(
                                    dst, ps[:],
                                    brep[:, oh * OHW:(oh + 1) * OHW])
                            else:
                                nc.any.tensor_add(dst, dst, ps[:])
                for b in range(B_BLK):
                    bs = blk * B_BLK + b
                    nc.sync.dma_start(out_d[bs * P:(bs + 1) * P, :], osbs[b][:])

    nc.compile()
    nc.finalize()
    return nc


_CACHE = {}
TRACE = False
LAST_EXEC_NS = None


def _get_nc():
    if "nc" not in _CACHE:
        _CACHE["nc"] = build_bass(4096, 4096, 512)
    return _CACHE["nc"]


def kernel(x, codes, absmax, bias):
    x = np.ascontiguousarray(np.asarray(x, dtype=np.float32))
    codes = np.ascontiguousarray(np.asarray(codes, dtype=np.int32))
    absmax = np.ascontiguousarray(np.asarray(absmax, dtype=np.float32))
    bias = np.ascontiguousarray(np.asarray(bias, dtype=np.float32))

    B, S, IN = x.shape
    OUT = codes.shape[0]
    BS = B * S
    OSH = OUT // N_CORES
    xf = np.ascontiguousarray(x.reshape(BS, IN))

    nc = _get_nc()
    in_maps = []
    for c in range(N_CORES):
        sl = slice(c * OSH, (c + 1) * OSH)
        in_maps.append({
            "x": xf,
            "codes": np.ascontiguousarray(codes[sl]),
            "absmax": np.ascontiguousarray(absmax[sl]),
            "bias": np.ascontiguousarray(bias[sl]),
        })
    global LAST_EXEC_NS
    res = run_bass_kernel_spmd(nc, in_maps, core_ids=list(range(N_CORES)),
                               trace=TRACE)
    LAST_EXEC_NS = res.exec_time_ns
    out = np.concatenate([res.results[c]["out"] for c in range(N_CORES)],
                         axis=1)
    return np.ascontiguousarray(out.reshape(B, S, OUT).astype(np.float32))
